# revision 10
# baseline (speedup 1.0000x reference)
"""Trainium2 Bass kernel for nn_Attention_XL (B=2,T=2048,C=1024,S=2048,H=16).

Sharding: (batch, head) pairs across 8 cores — core c handles batch c//4 and
heads [(c%4)*4, (c%4)*4+4). QKV projection is column-sharded by head (no
redundant FLOPs); W_proj is column-sharded; per-core partial outputs are
summed on the host (the tensor-parallel unshard step).

Per-core device program (everything stored feature-on-partition, i.e.
transposed):
  phase 1: qkv^T = W'^T.T @ q^T (fp32r matmuls), kcat^T = [k_xl^T+pos^T, kn^T].
  phase 1b: v_aug (v with a ones column for the softmax denominator) built via
           PE transposes of vn^T.
  phase 2: per head pair, flash-style loop over key chunks:
           scores^T chunk = kcat^T_chunk.T @ qn^T  (two heads packed in the
           128x128 PE array via row tiling, K=64 each, writing adjacent PSUM
           banks), exp on ACT (scale=1/sqrt(hd) folded in, one [128,1024]
           instruction covers both heads), out^T accumulated in PSUM via
           v_aug.T @ exp^T (ones column gives the softmax denominator in
           row 64), then normalize with a gpsimd-broadcast reciprocal.
  phase 3: out^T = Wp'.T @ y^T accumulated over heads in PSUM.
"""
import sys

sys.path.insert(0, "/opt/trn_rl_repo")

import numpy as np
import concourse.bass as bass
import concourse.bacc as bacc
import concourse.mybir as mybir
import concourse.tile as tile
from concourse.bass_utils import run_bass_kernel_spmd

F32 = mybir.dt.float32
F32R = mybir.dt.float32r
AF = mybir.ActivationFunctionType

HD = 64          # head dim
HPC = 4          # heads per core
N_CORES = 8


def r(ap):
    return ap.bitcast(F32R)


def build_program(T, S, C):
    """Build + compile the per-core Bass program. Same program on all cores."""
    L = S + T
    nL = L // 128           # key chunks
    nS = S // 128
    nT = T // 128
    nC = C // 128           # contraction chunks for qkv proj
    nTc = T // 512          # 512-wide t chunks
    R = 3 * HPC * HD        # rows of W' (768)
    nM = R // 128           # m-tiles of qkv output (6)
    scale = 1.0 / np.sqrt(HD)

    nc = bacc.Bacc("TRN2", target_bir_lowering=False, debug=False)

    qT = nc.dram_tensor("qT", [C, T], F32, kind="ExternalInput").ap()
    wqkvT = nc.dram_tensor("wqkvT", [C, R], F32, kind="ExternalInput").ap()
    wp4 = nc.dram_tensor("wp4", [HD, HPC * C], F32, kind="ExternalInput").ap()
    kxlT = nc.dram_tensor("kxlT", [HPC * HD, S], F32, kind="ExternalInput").ap()
    posT = nc.dram_tensor("posT", [HPC * HD, S], F32, kind="ExternalInput").ap()
    vxl = nc.dram_tensor("vxl", [S, HPC * HD], F32, kind="ExternalInput").ap()
    ident_in = nc.dram_tensor("ident", [128, 128], F32, kind="ExternalInput").ap()
    n_ones = max(64, 2 * (S + T) // 128)
    ones_in = nc.dram_tensor("ones", [128, n_ones], F32,
                             kind="ExternalInput").ap()
    outT = nc.dram_tensor("outT", [C, T], F32, kind="ExternalOutput").ap()

    with tile.TileContext(nc) as tc:
        import contextlib
        with contextlib.ExitStack() as ctx:
            # ---- pools that live for (nearly) the whole kernel ----
            persist = ctx.enter_context(tc.tile_pool(name="persist", bufs=1))
            vn_pool = ctx.enter_context(tc.tile_pool(name="vn_pool", bufs=1))
            tp_ps = ctx.enter_context(
                tc.tile_pool(name="tp_ps", bufs=1, space="PSUM"))

            ident = persist.tile([128, 128], F32, tag="ident")
            nc.sync.dma_start(ident[:], ident_in[:])
            ones_sb = persist.tile([128, 64], F32, tag="ones_sb")
            nc.sync.dma_start(r(ones_sb[:]), r(ones_in[:, 0:64]))

            qnT = [persist.tile([128, T], F32, tag=f"qnT{p}", name=f"qnT{p}")
                   for p in range(2)]
            kcatT = [persist.tile([128, L], F32, tag=f"kcatT{p}",
                                  name=f"kcatT{p}") for p in range(2)]
            yT = [persist.tile([64, T], F32, tag=f"yT{h}", name=f"yT{h}")
                  for h in range(HPC)]
            vnT = [vn_pool.tile([128, T], F32, tag=f"vnT{p}", name=f"vnT{p}")
                   for p in range(2)]

            # ---- phase 1: load q^T / W', project qkv, build kcat^T ----
            with tc.tile_pool(name="ph1", bufs=1) as ph1, \
                 tc.tile_pool(name="qkv_ps", bufs=4, space="PSUM") as qkv_ps:
                qt = ph1.tile([128, nC, T], F32, tag="qt")
                nc.sync.dma_start(
                    r(qt[:]), r(qT.rearrange("(n p) t -> p n t", p=128)))
                wq = ph1.tile([128, nC, R], F32, tag="wq")
                nc.sync.dma_start(
                    r(wq[:]), r(wqkvT.rearrange("(n p) m -> p n m", p=128)))

                # kx part of kcat^T: k_xl^T + pos^T  (DVE add, in place)
                for p in range(2):
                    pos_t = ph1.tile([128, S], F32, tag="pos", name="pos_t")
                    nc.sync.dma_start(
                        r(kcatT[p][:, 0:S]),
                        r(kxlT[p * 128:(p + 1) * 128, :]))
                    nc.sync.dma_start(pos_t[:], posT[p * 128:(p + 1) * 128, :])
                    nc.vector.tensor_add(
                        r(kcatT[p][:, 0:S]), kcatT[p][:, 0:S], pos_t[:])

                # qkv projection: m-tile destinations
                #   m=0,1 -> qn^T pair m ; m=2,3 -> kn^T into kcat^T ;
                #   m=4,5 -> vn^T (transposed into v_aug in phase 1b)
                for m in range(nM):
                    for t in range(nTc):
                        ts = slice(t * 512, (t + 1) * 512)
                        ps = qkv_ps.tile([128, 512], F32, tag="qkv")
                        for k in range(nC):
                            nc.tensor.matmul(
                                ps[:],
                                r(wq[:, k, m * 128:(m + 1) * 128]),
                                r(qt[:, k, ts]),
                                start=(k == 0), stop=(k == nC - 1))
                        p = m % 2
                        if m < 2:
                            dst = qnT[p][:, ts]
                        elif m < 4:
                            dst = kcatT[p][:, S + t * 512:S + (t + 1) * 512]
                        else:
                            dst = vnT[p][:, ts]
                        nc.vector.tensor_copy(r(dst), ps[:])

            # ---- phase 1b + 2: v_aug fill, then the attention flash loop ----
            with tc.tile_pool(name="att_sb", bufs=1) as att2, \
                 tc.tile_pool(name="sc_ps", bufs=2, space="PSUM") as sc_ps, \
                 tc.tile_pool(name="mm2_ps", bufs=1, space="PSUM") as mm2_ps, \
                 tc.tile_pool(name="bc_ps", bufs=1, space="PSUM") as bc_ps:
                v_aug = [att2.tile([128, nL * 66], F32, tag=f"vaug{h}",
                                   name=f"vaug{h}") for h in range(HPC)]
                for h in range(HPC):
                    va = v_aug[h].rearrange("p (n w) -> p n w", w=66)
                    nc.sync.dma_start(
                        r(va[:, :, 64:66]),
                        r(ones_in[:, 0:2 * nL]
                          .rearrange("p (n w) -> p n w", w=2)))
                    nc.sync.dma_start(
                        r(va[:, 0:nS, 0:HD]),
                        r(vxl.rearrange("(n p) d -> p n d", p=128)
                          [:, :, h * HD:(h + 1) * HD]))

                # vn^T -> v_aug via PE transpose ([128,128] blocks; two heads
                # per pair come out as columns 0:64 / 64:128)
                for p in range(2):
                    va0 = v_aug[2 * p].rearrange("p (n w) -> p n w", w=66)
                    va1 = v_aug[2 * p + 1].rearrange("p (n w) -> p n w", w=66)
                    for i in range(nT):
                        tp = tp_ps.tile([128, 128], F32, tag="tp")
                        nc.tensor.transpose(
                            tp[:], vnT[p][:, i * 128:(i + 1) * 128], ident[:])
                        nc.vector.tensor_copy(
                            r(va0[:, nS + i, 0:HD]), tp[:, 0:HD])
                        nc.vector.tensor_copy(
                            r(va1[:, nS + i, 0:HD]), tp[:, HD:128])

                for p in range(2):
                    hA, hB = 2 * p, 2 * p + 1
                    vaA = v_aug[hA].rearrange("p (n w) -> p n w", w=66)
                    vaB = v_aug[hB].rearrange("p (n w) -> p n w", w=66)
                    for t in range(nTc):
                        ts = slice(t * 512, (t + 1) * 512)
                        oA = mm2_ps.tile([66, 512], F32, tag="mm2A")
                        oB = mm2_ps.tile([66, 512], F32, tag="mm2B")
                        for l in range(nL):
                            lsl = slice(l * 128, (l + 1) * 128)
                            sc = sc_ps.tile([128, 1024], F32, tag="sc")
                            nc.tensor.matmul(
                                sc[:, 0:512],
                                r(kcatT[p][0:64, lsl]), r(qnT[p][0:64, ts]),
                                start=True, stop=True, tile_position=(0, 0))
                            nc.tensor.matmul(
                                sc[:, 512:1024],
                                r(kcatT[p][64:128, lsl]), r(qnT[p][64:128, ts]),
                                start=True, stop=True, tile_position=(64, 0))
                            et = att2.tile([128, 1024], F32, tag="exp", bufs=3)
                            nc.scalar.activation(r(et[:]), sc[:], AF.Exp,
                                                 scale=float(scale))
                            nc.tensor.matmul(
                                oA[:], r(vaA[:, l, :]), r(et[:, 0:512]),
                                start=(l == 0), stop=(l == nL - 1))
                            nc.tensor.matmul(
                                oB[:], r(vaB[:, l, :]), r(et[:, 512:1024]),
                                start=(l == 0), stop=(l == nL - 1))
                        # normalize: y^T = out^T * (1/sumexp), sumexp = row 64
                        for h, o in ((hA, oA), (hB, oB)):
                            srow = att2.tile([128, 512], F32, tag="srow",
                                             bufs=2)
                            nc.vector.tensor_copy(r(srow[64:65, :]),
                                                  o[64:65, :])
                            # broadcast sumexp row to 64 partitions via a
                            # K=1 matmul: ones[64] (x) srow
                            bc = bc_ps.tile([64, 512], F32, tag="bc")
                            nc.tensor.matmul(
                                bc[:], r(ones_sb[64:65, 0:64]),
                                r(srow[64:65, :]),
                                start=True, stop=True, tile_position=(64, 0))
                            brec = att2.tile([64, 512], F32, tag="brec", bufs=2)
                            nc.vector.reciprocal(brec[:], bc[:])
                            nc.vector.tensor_mul(
                                r(yT[h][0:64, ts]), o[0:64, :], brec[:])

            # ---- phase 3: projection out^T = Wp'.T @ y^T ----
            with tc.tile_pool(name="ph3", bufs=2) as ph3, \
                 tc.tile_pool(name="proj_ps", bufs=4, space="PSUM") as proj_ps:
                wp = ph3.tile([HD, HPC * C], F32, tag="wp", bufs=1)
                nc.sync.dma_start(r(wp[:]), r(wp4[:]))
                for d in range(nC):
                    ob = ph3.tile([128, T], F32, tag="ob")
                    for t in range(nTc):
                        ts = slice(t * 512, (t + 1) * 512)
                        ps = proj_ps.tile([128, 512], F32, tag="proj")
                        for h in range(HPC):
                            nc.tensor.matmul(
                                ps[:],
                                r(wp[0:64,
                                     h * C + d * 128:h * C + (d + 1) * 128]),
                                r(yT[h][0:64, ts]),
                                start=(h == 0), stop=(h == HPC - 1),
                                tile_position=(0, 0))
                        nc.vector.tensor_copy(ob[:, ts], ps[:])
                    nc.sync.dma_start(outT[d * 128:(d + 1) * 128, :], ob[:])

    nc.compile()
    return nc


_cache = {}


def _program(T, S, C):
    key = (T, S, C)
    if key not in _cache:
        _cache[key] = build_program(T, S, C)
    return _cache[key]


def core_inputs(q, k_xl, v_xl, W_qkv, W_proj, pos_emb, core):
    """Host-side shard prep for one core (slicing + layout transposes)."""
    C = q.shape[2]
    b = core // 4
    h0 = (core % 4) * HPC
    cols = slice(h0 * HD, (h0 + HPC) * HD)
    rows = np.r_[h0 * HD:(h0 + HPC) * HD]
    wrows = np.concatenate([rows, C + rows, 2 * C + rows])
    wp4 = (W_proj[:, cols].T.reshape(HPC, HD, C)
           .transpose(1, 0, 2).reshape(HD, HPC * C))
    return {
        "qT": np.ascontiguousarray(q[b].T),
        "wqkvT": np.ascontiguousarray(W_qkv[wrows].T),
        "wp4": np.ascontiguousarray(wp4),
        "kxlT": np.ascontiguousarray(k_xl[b].T[cols]),
        "posT": np.ascontiguousarray(pos_emb.T[cols]),
        "vxl": np.ascontiguousarray(v_xl[b][:, cols]),
        "ident": np.eye(128, dtype=np.float32),
        "ones": np.ones(
            (128, max(64, 2 * (q.shape[1] + k_xl.shape[1]) // 128)),
            np.float32),
    }


def kernel(q, k_xl, v_xl, W_qkv, W_proj, pos_emb, is_causal):
    q = np.asarray(q, np.float32)
    k_xl = np.asarray(k_xl, np.float32)
    v_xl = np.asarray(v_xl, np.float32)
    W_qkv = np.asarray(W_qkv, np.float32)
    W_proj = np.asarray(W_proj, np.float32)
    pos_emb = np.asarray(pos_emb, np.float32)
    B, T, C = q.shape
    S = k_xl.shape[1]

    nc = _program(T, S, C)
    in_maps = [core_inputs(q, k_xl, v_xl, W_qkv, W_proj, pos_emb, c)
               for c in range(N_CORES)]
    res = run_bass_kernel_spmd(nc, in_maps, list(range(N_CORES)))

    out = np.zeros((B, T, C), np.float32)
    for c in range(N_CORES):
        out[c // 4] += res.results[c]["outT"].T
    return out


# revision 21
# speedup vs baseline: 350.5133x; 350.5133x over previous
"""Trainium2 Bass kernel for nn_Attention_XL (B=2,T=2048,C=1024,S=2048,H=16).

Sharding: (batch, head) pairs across 8 cores — core c handles batch c//4 and
heads [(c%4)*4, (c%4)*4+4). QKV projection is column-sharded by head (no
redundant FLOPs); W_proj is column-sharded; per-core partial outputs are
summed on the host (the tensor-parallel unshard step).

Per-core device program (everything stored feature-on-partition, i.e.
transposed; all matmuls fp32r):
  kcat^T = [k_xl^T (+)DMA-accumulated pos^T, kn^T]; vn computed directly in
  [t, dim] layout (one N=256 matmul chain covers all 4 heads) into v_aug,
  whose ones column later yields the softmax denominator.
  Attention per head pair, flash-style over key chunks: scores^T via
  row-tiled K=64 matmuls (two heads packed in the PE array, adjacent PSUM
  banks), one [128,1024] ACT exp per chunk (scale folded in), out^T
  accumulated in PSUM, normalization via a K=1-matmul broadcast of
  1/sumexp. Pair-0's attention is emitted before pair-1's QKV m-tiles so
  the ACT-bound loop overlaps the remaining projection work; the output
  projection is drip-fed into pair-1's l-loops; each normalize is delayed
  past the next t-chunk's first scores.
"""
import sys

sys.path.insert(0, "/opt/trn_rl_repo")

import numpy as np
import concourse.bass as bass
import concourse.bacc as bacc
import concourse.mybir as mybir
import concourse.tile as tile
from concourse.bass_utils import run_bass_kernel_spmd

F32 = mybir.dt.float32
F32R = mybir.dt.float32r
AF = mybir.ActivationFunctionType
ADD = mybir.AluOpType.add

HD = 64          # head dim
HPC = 4          # heads per core
N_CORES = 8


def r(ap):
    return ap.bitcast(F32R)


def build_program(T, S, C):
    """Build + compile the per-core Bass program. Same program on all cores."""
    L = S + T
    nL = L // 128           # key chunks
    nS = S // 128
    nT = T // 128
    nC = C // 128           # contraction chunks for qkv proj
    nTc = T // 512          # 512-wide t chunks
    R = 3 * HPC * HD        # rows of W' (768)
    scale = 1.0 / np.sqrt(HD)

    nc = bacc.Bacc("TRN2", target_bir_lowering=False, debug=False)

    qT = nc.dram_tensor("qT", [C, T], F32, kind="ExternalInput").ap()
    wqkvT = nc.dram_tensor("wqkvT", [C, R], F32, kind="ExternalInput").ap()
    wp4 = nc.dram_tensor("wp4", [HD, HPC * C], F32, kind="ExternalInput").ap()
    kxlT = nc.dram_tensor("kxlT", [HPC * HD, S], F32, kind="ExternalInput").ap()
    posT = nc.dram_tensor("posT", [HPC * HD, S], F32, kind="ExternalInput").ap()
    vxl = nc.dram_tensor("vxl", [S, HPC * HD], F32, kind="ExternalInput").ap()
    n_ones = max(64, 2 * (S + T) // 128)
    ones_in = nc.dram_tensor("ones", [128, n_ones], F32,
                             kind="ExternalInput").ap()
    outT = nc.dram_tensor("outT", [C, T], F32, kind="ExternalOutput").ap()

    with tile.TileContext(nc) as tc:
        import contextlib
        with contextlib.ExitStack() as ctx:
            persist = ctx.enter_context(tc.tile_pool(name="persist", bufs=1))
            vaugp = ctx.enter_context(tc.tile_pool(name="vaugp", bufs=1))
            att2 = ctx.enter_context(tc.tile_pool(name="att_sb", bufs=1))

            ones_sb = persist.tile([128, 64], F32, tag="ones_sb")
            qnT = [persist.tile([128, T], F32, tag=f"qnT{p}", name=f"qnT{p}")
                   for p in range(2)]
            kcatT = [persist.tile([128, L], F32, tag=f"kcatT{p}",
                                  name=f"kcatT{p}") for p in range(2)]
            yT = [persist.tile([64, T], F32, tag=f"yT{h}", name=f"yT{h}")
                  for h in range(HPC)]
            v_aug = [vaugp.tile([128, nL * 66], F32, tag=f"vaug{h}",
                                name=f"vaug{h}") for h in range(HPC)]

            # psum pools for the attention loops are entered mid-build
            # (after the big qkv pool closes); declared here for closures
            sc_ps = mm2_ps = bc_ps = pj_ps = None
            proj_group = None

            def normalize(hA, hB, oA, oB, ts):
                # y^T = out^T * (1/sumexp), sumexp in row 64
                for h, o in ((hA, oA), (hB, oB)):
                    srow = att2.tile([128, 512], F32, tag="srow",
                                     bufs=1, name="srow")
                    nc.vector.tensor_copy(r(srow[64:65, :]), o[64:65, :])
                    bc = bc_ps.tile([64, 512], F32, tag="bc", name="bc")
                    nc.tensor.matmul(
                        bc[:], r(ones_sb[64:65, 0:64]), r(srow[64:65, :]),
                        start=True, stop=True, tile_position=(64, 0))
                    brec = att2.tile([64, 512], F32, tag="brec",
                                     bufs=2, name="brec")
                    nc.vector.reciprocal(brec[:], bc[:])
                    nc.vector.tensor_mul(
                        r(yT[h][0:64, ts]), o[0:64, :], brec[:])

            def attention(p, proj_feed, pending, extra=None):
                hA, hB = 2 * p, 2 * p + 1
                vaA = v_aug[hA].rearrange("p (n w) -> p n w", w=66)
                vaB = v_aug[hB].rearrange("p (n w) -> p n w", w=66)
                for t in range(nTc):
                    ts = slice(t * 512, (t + 1) * 512)
                    oA = mm2_ps.tile([66, 512], F32, tag="mm2A", name="oA")
                    oB = mm2_ps.tile([66, 512], F32, tag="mm2B", name="oB")
                    feed = proj_feed(t) if proj_feed else []
                    for l in range(nL):
                        if extra:
                            extra(t, l)
                        lsl = slice(l * 128, (l + 1) * 128)
                        sc = sc_ps.tile([128, 1024], F32, tag="sc", name="sc")
                        nc.tensor.matmul(
                            sc[:, 0:512],
                            r(kcatT[p][0:64, lsl]), r(qnT[p][0:64, ts]),
                            start=True, stop=True, tile_position=(0, 0))
                        nc.tensor.matmul(
                            sc[:, 512:1024],
                            r(kcatT[p][64:128, lsl]), r(qnT[p][64:128, ts]),
                            start=True, stop=True, tile_position=(64, 0))
                        et = att2.tile([128, 1024], F32, tag="exp",
                                       bufs=2, name="et")
                        nc.scalar.activation(r(et[:]), sc[:], AF.Exp,
                                             scale=float(scale))
                        if l == 4 and pending:
                            normalize(*pending)
                            pending = None
                        nc.tensor.matmul(
                            oA[:], r(vaA[:, l, :]), r(et[:, 0:512]),
                            start=(l == 0), stop=(l == nL - 1))
                        nc.tensor.matmul(
                            oB[:], r(vaB[:, l, :]), r(et[:, 512:1024]),
                            start=(l == 0), stop=(l == nL - 1))
                        if feed and l >= 6 and (l - 6) % 3 == 0:
                            d = (l - 6) // 3
                            if d < len(feed):
                                proj_group(*feed[d])
                    pending = (hA, hB, oA, oB, ts)
                return pending

            # ---- phase 1: loads + qkv projection ----
            with tc.tile_pool(name="ph1", bufs=1) as ph1:
                qTr = qT.rearrange("(n p) t -> p n t", p=128)
                wqr = wqkvT.rearrange("(n p) m -> p n m", p=128)
                # DMA order = criticality: W(m=0), q^T, pair-0 k/v loads
                wqm0 = ph1.tile([128, nC, 128], F32, tag="wqm0")
                nc.sync.dma_start(r(wqm0[:]), r(wqr[:, :, 0:128]))
                qt = ph1.tile([128, nC, T], F32, tag="qt")
                for k in range(nC):
                    nc.sync.dma_start(r(qt[:, k:k + 1, :]),
                                      r(qTr[:, k:k + 1, :]))

                def load_pair(p):
                    # kcat^T kx part (pos^T DMA-accumulated) + v_aug XL part
                    nc.sync.dma_start(
                        r(kcatT[p][:, 0:S]),
                        r(kxlT[p * 128:(p + 1) * 128, :]))
                    nc.gpsimd.dma_start(
                        r(kcatT[p][:, 0:S]),
                        r(posT[p * 128:(p + 1) * 128, :]),
                        accum_op=ADD)
                    for h in (2 * p, 2 * p + 1):
                        va = v_aug[h].rearrange("p (n w) -> p n w", w=66)
                        nc.sync.dma_start(
                            r(va[:, :, 64:66]),
                            r(ones_in[:, 0:2 * nL]
                              .rearrange("p (n w) -> p n w", w=2)))
                        nc.sync.dma_start(
                            r(va[:, 0:nS, 0:HD]),
                            r(vxl.rearrange("(n p) d -> p n d", p=128)
                              [:, :, h * HD:(h + 1) * HD]))

                load_pair(0)
                nc.sync.dma_start(r(ones_sb[:]), r(ones_in[:, 0:64]))
                wq45 = ph1.tile([128, nC, 2 * 128], F32, tag="wq45")
                nc.sync.dma_start(r(wq45[:]), r(wqr[:, :, 512:768]))

                def qkv_m(m, pool, wqm=None):
                    # one 128-row m-tile of the qkv projection
                    if wqm is None:
                        wqm = ph1.tile([128, nC, 128], F32, tag="wqm",
                                       bufs=1, name="wqm")
                        nc.sync.dma_start(
                            r(wqm[:]), r(wqr[:, :, m * 128:(m + 1) * 128]))
                    p = m % 2
                    for t in range(nTc):
                        ts = slice(t * 512, (t + 1) * 512)
                        ps = pool.tile([128, 512], F32, tag="qkv", name="ps")
                        for k in range(nC):
                            nc.tensor.matmul(
                                ps[:], r(wqm[:, k, :]), r(qt[:, k, ts]),
                                start=(k == 0), stop=(k == nC - 1))
                        dst = (qnT[p][:, ts] if m < 2 else
                               kcatT[p][:, S + t * 512:S + (t + 1) * 512])
                        nc.vector.tensor_copy(r(dst), ps[:])

                with tc.tile_pool(name="qkv1", bufs=3, space="PSUM") as qkv1:
                    qkv_m(0, qkv1, wqm=wqm0)   # qn^T pair 0
                    qkv_m(2, qkv1)   # kn^T pair 0

                # attention psum pools (outlive ph1; LIFO within PSUM is
                # independent of the SBUF pool stack)
                sc_ps = ctx.enter_context(
                    tc.tile_pool(name="sc_ps", bufs=2, space="PSUM"))
                mm2_ps = ctx.enter_context(
                    tc.tile_pool(name="mm2_ps", bufs=1, space="PSUM"))
                bc_ps = ctx.enter_context(
                    tc.tile_pool(name="bc_ps", bufs=1, space="PSUM"))

                # vn for all 4 heads, directly in [t, dim] layout, computed
                # chunk-by-chunk inside pair-0 tchunk-0's l-loop: group j is
                # written at iteration j and first read at iteration 16+j
                with tc.tile_pool(name="vnp", bufs=1, space="PSUM") as vnp:
                    def vn_extra(t, l):
                        if t != 0 or l >= nT:
                            return
                        i = l
                        vn = vnp.tile([128, 256], F32, tag="vn", name="vn")
                        for k in range(nC):
                            nc.tensor.matmul(
                                vn[:],
                                r(qt[:, k, i * 128:(i + 1) * 128]),
                                r(wq45[:, k, :]),
                                start=(k == 0), stop=(k == nC - 1))
                        for h in range(HPC):
                            va = v_aug[h].rearrange("p (n w) -> p n w", w=66)
                            nc.vector.tensor_copy(
                                r(va[:, nS + i, 0:HD]),
                                vn[:, h * HD:(h + 1) * HD])

                    # pair-0 attention; remaining qkv m-tiles emitted after
                    # it fill PE slack under the ACT-bound loop
                    pending = attention(0, None, None, extra=vn_extra)

                load_pair(1)
                with tc.tile_pool(name="qkv2", bufs=1, space="PSUM") as qkv2:
                    qkv_m(1, qkv2)   # qn^T pair 1
                    qkv_m(3, qkv2)   # kn^T pair 1

            # ---- pair-1 attention with drip-fed output projection ----
            with tc.tile_pool(name="tail_sb", bufs=1) as tail, \
                 tc.tile_pool(name="pj_ps", bufs=1, space="PSUM") as pj_ps:
                wp = tail.tile([HD, HPC * C], F32, tag="wp")
                nc.sync.dma_start(r(wp[:]), r(wp4[:]))

                def proj_group(t, d):
                    # out^T[d-chunk, tchunk t] over all 4 heads
                    ts = slice(t * 512, (t + 1) * 512)
                    ps = pj_ps.tile([128, 512], F32, tag="proj", name="pj")
                    for h in range(HPC):
                        nc.tensor.matmul(
                            ps[:],
                            r(wp[0:64, h * C + d * 128:h * C + (d + 1) * 128]),
                            r(yT[h][0:64, ts]),
                            start=(h == 0), stop=(h == HPC - 1),
                            tile_position=(0, 0))
                    ob = tail.tile([128, 512], F32, tag="ob", bufs=4,
                                   name="ob")
                    nc.vector.tensor_copy(ob[:], ps[:])
                    nc.sync.dma_start(outT[d * 128:(d + 1) * 128, ts], ob[:])

                def feed(t):
                    # during pair-1 tchunk t, project tchunk t-1
                    if t == 0:
                        return []
                    return [(t - 1, d) for d in range(nC)]

                pending = attention(1, feed, pending)
                normalize(*pending)
                for d in range(nC):
                    proj_group(nTc - 1, d)

    nc.compile()
    return nc


_cache = {}


def _program(T, S, C):
    key = (T, S, C)
    if key not in _cache:
        _cache[key] = build_program(T, S, C)
    return _cache[key]


def core_inputs(q, k_xl, v_xl, W_qkv, W_proj, pos_emb, core):
    """Host-side shard prep for one core (slicing + layout transposes)."""
    C = q.shape[2]
    b = core // 4
    h0 = (core % 4) * HPC
    cols = slice(h0 * HD, (h0 + HPC) * HD)
    rows = np.r_[h0 * HD:(h0 + HPC) * HD]
    wrows = np.concatenate([rows, C + rows, 2 * C + rows])
    wp4 = (W_proj[:, cols].T.reshape(HPC, HD, C)
           .transpose(1, 0, 2).reshape(HD, HPC * C))
    return {
        "qT": np.ascontiguousarray(q[b].T),
        "wqkvT": np.ascontiguousarray(W_qkv[wrows].T),
        "wp4": np.ascontiguousarray(wp4),
        "kxlT": np.ascontiguousarray(k_xl[b].T[cols]),
        "posT": np.ascontiguousarray(pos_emb.T[cols]),
        "vxl": np.ascontiguousarray(v_xl[b][:, cols]),
        "ones": np.ones(
            (128, max(64, 2 * (q.shape[1] + k_xl.shape[1]) // 128)),
            np.float32),
    }


def kernel(q, k_xl, v_xl, W_qkv, W_proj, pos_emb, is_causal):
    q = np.asarray(q, np.float32)
    k_xl = np.asarray(k_xl, np.float32)
    v_xl = np.asarray(v_xl, np.float32)
    W_qkv = np.asarray(W_qkv, np.float32)
    W_proj = np.asarray(W_proj, np.float32)
    pos_emb = np.asarray(pos_emb, np.float32)
    B, T, C = q.shape
    S = k_xl.shape[1]

    nc = _program(T, S, C)
    in_maps = [core_inputs(q, k_xl, v_xl, W_qkv, W_proj, pos_emb, c)
               for c in range(N_CORES)]
    res = run_bass_kernel_spmd(nc, in_maps, list(range(N_CORES)))

    out = np.zeros((B, T, C), np.float32)
    for c in range(N_CORES):
        out[c // 4] += res.results[c]["outT"].T
    return out
